# revision 1
# baseline (speedup 1.0000x reference)
"""Conv1d (B=32, C_in=C_out=256, W=4096, K=3, pad=1) on 8 Trainium2 cores.

Strategy: data-parallel over batch (4 per core). Per core the conv is a sum
of 6 accumulated matmuls per 512-position output chunk: contraction over
(tap u in 0..2, ci_chunk in 0..1) with lhsT = weight[ci_chunk, :, co_chunk,
u].T ([128 ci x 128 co]) and rhs = a padded-x slice [128 ci x 512]. fp16
inputs (same PE rate as bf16, 8x lower error), fp32 PSUM accumulation, bias
added during the PSUM->SBUF drain on DVE.

Layout/scheduling choices (measured on HW):
- x arrives as quarter tiles (separate tiles - Tile tracks SBUF deps per
  whole tile) on the ACT HWDGE ring, ci-interleaved for batch 0, so the
  first matmuls start right after the ~7us framework prologue and PE never
  stalls on input data.
- each PSUM bank accumulates one chunk's 6 matmuls, then DVE drains it
  with the bias add; 8 banks cycle so PE streams back-to-back.
- outputs staged per (b, co) and flushed per finished 1024-position
  quarter so the tail only waits on a 0.5MB store.
"""

import numpy as np

F16 = np.float16

B, C, W, K = 32, 256, 4096, 3
NCORES = 8
BPC = B // NCORES          # batches per core
P = 128                    # partitions
CIC = C // P               # ci chunks
COC = C // P               # co chunks
NCH = 512                  # positions per matmul (one PSUM bank of fp32)
NCHUNKS = W // NCH         # position chunks per batch row
NQ = 4                     # x quarter tiles (batch 0)
QW = W // NQ               # 1024 positions per quarter

_cache = {}


def _build_program():
    import concourse.bass as bass
    import concourse.bacc as bacc
    import concourse.mybir as mybir
    from concourse import tile

    nc = bacc.Bacc(None, target_bir_lowering=False)
    # x, padded by one position on each side, pre-split in quarters with a
    # 2-column overlap: xq[b, ci, q] covers padded columns q*QW .. q*QW+QW+1.
    xq_d = nc.dram_tensor("xq", [BPC, CIC, NQ, P, QW + 2], mybir.dt.float16,
                          kind="ExternalInput")
    w_d = nc.dram_tensor("wt", [P, K * CIC * COC, P], mybir.dt.float16,
                         kind="ExternalInput")
    b_d = nc.dram_tensor("bb", [P, COC], mybir.dt.float32,
                         kind="ExternalInput")
    out_d = nc.dram_tensor("out", [BPC, COC, P, W], mybir.dt.float32,
                           kind="ExternalOutput")

    with tile.TileContext(nc) as tc:
        with (
            tc.tile_pool(name="wp", bufs=1) as wp,
            tc.tile_pool(name="xpool", bufs=BPC * CIC * NQ) as xpool,
            tc.tile_pool(name="opool", bufs=3) as opool,
            tc.tile_pool(name="pspool", bufs=8, space=bass.MemorySpace.PSUM) as pspool,
        ):
            w_sb = wp.tile([P, K * CIC * COC, P], mybir.dt.float16)
            nc.sync.dma_start(w_sb[:], w_d[:])
            b_sb = wp.tile([P, COC], mybir.dt.float32)
            nc.sync.dma_start(b_sb[:], b_d[:])

            # x quarter tiles; batch 0 first (quarter by quarter, ci
            # interleaved), then batches 1-3 with one DMA per (b, ci, q).
            x_sb = {}
            for b in range(BPC):
                for ci in range(CIC):
                    for q in range(NQ):
                        x_sb[(b, ci, q)] = xpool.tile(
                            [P, QW + 2], mybir.dt.float16,
                            name=f"xt_{b}_{ci}_{q}", tag="xt")
            for q in range(NQ):
                for ci in range(CIC):
                    nc.scalar.dma_start(x_sb[(0, ci, q)][:], xq_d[0, ci, q])
            for b in range(1, BPC):
                for ci in range(CIC):
                    for q in range(NQ):
                        nc.scalar.dma_start(x_sb[(b, ci, q)][:], xq_d[b, ci, q])

            def rhs(b, ci, n, u):
                # positions n*NCH .. n*NCH+511, tap offset u -> padded
                # columns n*NCH+u .. ; quarter q holds padded cols
                # q*QW .. q*QW+QW+1 at local offset -q*QW.
                q = (n * NCH) // QW
                lo = n * NCH + u - q * QW
                return x_sb[(b, ci, q)][:, lo:lo + NCH]

            NACC = K * CIC
            for b in range(BPC):
                for co in range(COC):
                    o_sb = opool.tile([P, W], mybir.dt.float32)
                    for n in range(NCHUNKS):
                        ps = pspool.tile([P, NCH], mybir.dt.float32,
                                         name=f"ps_{b}_{co}_{n}", tag="ps")
                        for k, (u, ci) in enumerate(
                                (u, ci) for u in range(K) for ci in range(CIC)):
                            nc.tensor.matmul(
                                ps[:], w_sb[:, (u * CIC + ci) * COC + co, :],
                                rhs(b, ci, n, u),
                                start=(k == 0), stop=(k == NACC - 1),
                            )
                        nc.vector.tensor_scalar_add(
                            o_sb[:, n * NCH:(n + 1) * NCH], ps[:],
                            b_sb[:, co:co + 1],
                        )
                        if n % 2 == 1:  # flush each finished quarter
                            qq = n // 2
                            nc.sync.dma_start(
                                out_d[b, co, :, qq * QW:(qq + 1) * QW],
                                o_sb[:, qq * QW:(qq + 1) * QW])
    nc.compile()
    return nc


def _prep_inputs(x, weight, bias):
    # x: [32,256,4096] f32 -> padded fp16 quarters [B, CIC, NQ, 128, QW+2]
    xp = np.zeros((B, CIC, P, W + 2), F16)
    xp[:, :, :, 1:W + 1] = x.reshape(B, CIC, P, W).astype(F16)
    xq = np.empty((B, CIC, NQ, P, QW + 2), F16)
    for q in range(NQ):
        xq[:, :, q] = xp[:, :, :, q * QW:q * QW + QW + 2]
    # weight: [co, ci, u] -> [ci_in, (u, ci_c, co_c), co_in]
    wt = weight.reshape(COC, P, CIC, P, K)          # [co_c, co_in, ci_c, ci_in, u]
    w_host = np.ascontiguousarray(
        wt.transpose(3, 4, 2, 0, 1)                 # [ci_in, u, ci_c, co_c, co_in]
    ).reshape(P, K * CIC * COC, P).astype(F16)
    b_host = np.ascontiguousarray(bias.reshape(COC, P).T).astype(np.float32)
    return xq, w_host, b_host


def run(x, weight, bias, trace=False):
    from concourse.bass_utils import run_bass_kernel_spmd

    if "nc" not in _cache:
        _cache["nc"] = _build_program()
    nc = _cache["nc"]

    xq, w_host, b_host = _prep_inputs(
        np.asarray(x, np.float32), np.asarray(weight, np.float32),
        np.asarray(bias, np.float32))
    in_maps = [
        {"xq": xq[c * BPC:(c + 1) * BPC], "wt": w_host, "bb": b_host}
        for c in range(NCORES)
    ]
    res = run_bass_kernel_spmd(nc, in_maps, list(range(NCORES)), trace=trace)
    out = np.concatenate(
        [res.results[c]["out"].reshape(BPC, C, W) for c in range(NCORES)], axis=0)
    return out, res


def kernel(x, weight, bias):
    out, _ = run(x, weight, bias, trace=False)
    return out



# revision 3
# speedup vs baseline: 1.0588x; 1.0588x over previous
"""Conv1d (B=32, C_in=C_out=256, W=4096, K=3, pad=1) on 8 Trainium2 cores.

Strategy: data-parallel over batch (4 per core). Per core the conv is 6
accumulated matmuls per 512-position output chunk: contraction over (tap u,
ci_chunk) with lhsT = weight tile [128 ci x 128 co] and rhs = a padded-x
column block [128 ci x 512+2]. fp16 inputs, fp32 PSUM accumulation, bias
added during the PSUM->SBUF drain on DVE with fp16 output staging (halves
store bytes; host casts back to fp32).

v2 schedule (vs the 104.5us baseline): the old kernel's first matmul waited
13us for a 263KB x quarter on one ring and the tail spent ~6us storing a
512KB fp32 quarter. Now:
- x arrives as 64 per-chunk column blocks [128, 514] so each chunk's
  matmuls depend on exactly one small tile; the first blocks of batch 0 are
  the first DMAs issued on the sync ring, weights (12 separate [128,128]
  tiles, co-chunk 0 first) right behind them, so PE starts ~4us in.
- loads are spread over the sync/scalar/gpsimd/vector rings in consumption
  order (b0 -> b1 -> b2/b3); stores go on the sync ring once its loads are
  done issuing.
- outputs are staged per (b, co, quarter) [128, 1024] fp16 tiles and
  flushed as soon as both chunks drain; the final quarter is stored as two
  512-col pieces so the tail only waits on a 128KB transfer.
"""

import numpy as np

F16 = np.float16

B, C, W, K = 32, 256, 4096, 3
NCORES = 8
BPC = B // NCORES          # batches per core
P = 128                    # partitions
CIC = C // P               # ci chunks
COC = C // P               # co chunks
NCH = 512                  # positions per matmul (one PSUM bank of fp32)
NCHUNKS = W // NCH         # position chunks per batch row
BW = NCH + 2               # x block width (512 cols + 2-tap halo)
QW = 1024                  # store quarter width

_cache = {}


def _build_program():
    import concourse.bass as bass
    import concourse.bacc as bacc
    import concourse.mybir as mybir
    from concourse import tile

    nc = bacc.Bacc(None, target_bir_lowering=False)
    # x, padded by one position on each side, pre-split into NCHUNKS column
    # blocks with a 2-column halo: xb[b, ci, n] covers padded columns
    # n*512 .. n*512+513.
    xb_d = nc.dram_tensor("xb", [BPC, CIC, NCHUNKS, P, BW], mybir.dt.float16,
                          kind="ExternalInput")
    # weight tiles, t = coc*6 + u*CIC + cic, each [ci_in, co_in]
    w_d = nc.dram_tensor("wt", [K * CIC * COC, P, P], mybir.dt.float16,
                         kind="ExternalInput")
    b_d = nc.dram_tensor("bb", [P, COC], mybir.dt.float32,
                         kind="ExternalInput")
    out_d = nc.dram_tensor("out", [BPC, COC, P, W], mybir.dt.float16,
                           kind="ExternalOutput")

    with tile.TileContext(nc) as tc:
        with (
            tc.tile_pool(name="wp", bufs=K * CIC * COC + 1) as wp,
            tc.tile_pool(name="xpool", bufs=BPC * CIC * NCHUNKS) as xpool,
            tc.tile_pool(name="opool", bufs=6) as opool,
            tc.tile_pool(name="pspool", bufs=8, space=bass.MemorySpace.PSUM) as pspool,
        ):
            x_sb = {}
            for b in range(BPC):
                for ci in range(CIC):
                    for n in range(NCHUNKS):
                        x_sb[(b, ci, n)] = xpool.tile(
                            [P, BW], mybir.dt.float16,
                            name=f"xt_{b}_{ci}_{n}", tag="xt")
            w_sb = [wp.tile([P, P], mybir.dt.float16, name=f"wt_{t}", tag="wt")
                    for t in range(K * CIC * COC)]
            b_sb = wp.tile([P, COC], mybir.dt.float32)

            # -- load schedule --------------------------------------------
            # sync ring: the two blocks the first matmuls need, then the
            # weight tiles (co-chunk 0 group first) and bias.
            nc.sync.dma_start(x_sb[(0, 0, 0)][:], xb_d[0, 0, 0])
            nc.sync.dma_start(x_sb[(0, 1, 0)][:], xb_d[0, 1, 0])
            for t in range(K * CIC * COC):
                nc.sync.dma_start(w_sb[t][:], w_d[t])
            nc.sync.dma_start(b_sb[:], b_d[:])
            # scalar ring: rest of batch 0 (chunk-major), then batch 3.
            for n in range(1, NCHUNKS):
                for ci in range(CIC):
                    nc.scalar.dma_start(x_sb[(0, ci, n)][:], xb_d[0, ci, n])
            for n in range(NCHUNKS):
                for ci in range(CIC):
                    nc.scalar.dma_start(x_sb[(3, ci, n)][:], xb_d[3, ci, n])
            # gpsimd ring: batches 1 and 2 (only SP/ACT/gpsimd can issue DMA)
            for b in (1, 2):
                for n in range(NCHUNKS):
                    for ci in range(CIC):
                        nc.gpsimd.dma_start(x_sb[(b, ci, n)][:], xb_d[b, ci, n])

            # -- compute --------------------------------------------------
            NACC = K * CIC
            last = (BPC - 1, COC - 1)
            for b in range(BPC):
                for co in range(COC):
                    for n in range(NCHUNKS):
                        q, h = n // 2, n % 2
                        if h == 0:
                            o_sb = opool.tile([P, QW], mybir.dt.float16,
                                              name=f"ot_{b}_{co}_{q}", tag="ot")
                        ps = pspool.tile([P, NCH], mybir.dt.float32,
                                         name=f"ps_{b}_{co}_{n}", tag="ps")
                        for k, (u, ci) in enumerate(
                                (u, ci) for u in range(K) for ci in range(CIC)):
                            nc.tensor.matmul(
                                ps[:], w_sb[co * NACC + u * CIC + ci][:],
                                x_sb[(b, ci, n)][:, u:u + NCH],
                                start=(k == 0), stop=(k == NACC - 1),
                            )
                        nc.vector.tensor_scalar_add(
                            o_sb[:, h * NCH:(h + 1) * NCH], ps[:],
                            b_sb[:, co:co + 1],
                        )
                        if (b, co) == last and q == 3:
                            # tail: flush the final quarter per chunk
                            nc.sync.dma_start(
                                out_d[b, co, :, n * NCH:(n + 1) * NCH],
                                o_sb[:, h * NCH:(h + 1) * NCH])
                        elif h == 1:
                            nc.sync.dma_start(
                                out_d[b, co, :, q * QW:(q + 1) * QW], o_sb[:])
    nc.compile()
    return nc


def _prep_inputs(x, weight, bias):
    # x: [32,256,4096] f32 -> padded fp16 blocks [B, CIC, NCHUNKS, 128, 514]
    xp = np.zeros((B, CIC, P, W + 2), F16)
    xp[:, :, :, 1:W + 1] = x.reshape(B, CIC, P, W).astype(F16)
    xb = np.empty((B, CIC, NCHUNKS, P, BW), F16)
    for n in range(NCHUNKS):
        xb[:, :, n] = xp[:, :, :, n * NCH:n * NCH + BW]
    # weight: [co, ci, u] -> tiles [coc*6 + u*CIC + cic][ci_in, co_in]
    wt = weight.reshape(COC, P, CIC, P, K)          # [coc, co_in, cic, ci_in, u]
    w_host = np.ascontiguousarray(
        wt.transpose(0, 4, 2, 3, 1)                 # [coc, u, cic, ci_in, co_in]
    ).reshape(K * CIC * COC, P, P).astype(F16)
    b_host = np.ascontiguousarray(bias.reshape(COC, P).T).astype(np.float32)
    return xb, w_host, b_host


def run(x, weight, bias, trace=False):
    from concourse.bass_utils import run_bass_kernel_spmd

    if "nc" not in _cache:
        _cache["nc"] = _build_program()
    nc = _cache["nc"]

    xb, w_host, b_host = _prep_inputs(
        np.asarray(x, np.float32), np.asarray(weight, np.float32),
        np.asarray(bias, np.float32))
    in_maps = [
        {"xb": xb[c * BPC:(c + 1) * BPC], "wt": w_host, "bb": b_host}
        for c in range(NCORES)
    ]
    res = run_bass_kernel_spmd(nc, in_maps, list(range(NCORES)), trace=trace)
    out = np.concatenate(
        [res.results[c]["out"].reshape(BPC, C, W) for c in range(NCORES)],
        axis=0).astype(np.float32)
    return out, res


def kernel(x, weight, bias):
    out, _ = run(x, weight, bias, trace=False)
    return out
